# revision 20
# baseline (speedup 1.0000x reference)
"""Trainium2 Bass kernel for single-query pooling attention.

Reference computation (B=32, N=4096, C=768, H=8, DH=96):
    q = (queries @ Wq.T).reshape(H, DH)
    k/v from x @ Wkv.T ; dots = q.k ; attn = softmax_n(dots)
    out = Wproj(attn-weighted sum of v) + bproj     -> [B, 1, C]

Algebraic reduction (never materializes k/v):
    wk_eff[h,c] = sum_d q[h,d] * Wkv[h*DH+d, c]         (host, tiny)
    dots[n,h]   = x[n,:] @ wk_eff[h,:]                  (PE, from xT)
    w = exp(dots)  (no max subtraction: |dots| <~ 20, safe in f32)
    pooled[h,c] = sum_n w[n,h] x[n,c] ; sumw[h] = sum_n w[n,h]
    z[hd]  = per-head pooled @ Wv.T ; out = z @ Wproj.T + bproj

v2: x is loaded from HBM ONCE (natural layout, bf16) — the kernel is
HBM-bound and the old two-layout scheme paid 2x bytes.  The transposed
view needed by the dots matmul is manufactured on-chip with PE
transposes (LDWEIGHTS streams x through the weight port while the
moving port replays an identity).  The dots and pooled matmuls have
only 8-wide outputs, so they are column-tiled: 4 concurrent 32-column
PE groups, one per 128-row subtile, giving ~4x effective moving
bandwidth.  The 4 partial pooled accumulators (one per column group)
are merged on the DVE at batch end.

Sharding: pure data-parallel over batch, 4 batches per core, 8 cores.
No collectives needed.
"""

import sys

sys.path.insert(0, "/opt/trn_rl_repo")

import numpy as np

import concourse.bass as bass
import concourse.tile as tile
from concourse import bacc, mybir

B, N, C, H = 32, 4096, 768, 8
DH = C // H
N_CORES = 8
B_LOC = B // N_CORES          # 4 batches per core
DTILE = 1024                  # rows per DMA transfer
DSUB = DTILE // 128           # 8 sub-rows per partition per DMA tile
PTILE = 512                   # rows per processing tile
NPT = N // PTILE              # 8 processing tiles per batch
CJ = C // 128                 # 6 c-chunks
C2 = C + 2                    # x padded with 2 ones columns
GRP = 4                       # column-tile groups (32-partition each)


def build_graph():
    cdt = mybir.dt.bfloat16
    f32 = mybir.dt.float32

    nc = bacc.Bacc("TRN2", target_bir_lowering=False, debug=False)

    x_d = nc.declare_dram_parameter("x", [B_LOC * N, C2], cdt, isOutput=False)
    wk_d = nc.declare_dram_parameter("wkT", [C, 32], cdt, isOutput=False)
    wv_d = nc.declare_dram_parameter("wvT", [C, C], cdt, isOutput=False)
    wp_d = nc.declare_dram_parameter("wpT", [C, C], cdt, isOutput=False)
    bp_d = nc.declare_dram_parameter("bproj", [C], f32, isOutput=False)
    id_d = nc.declare_dram_parameter("ident", [128, 128], cdt, isOutput=False)
    id4_d = nc.declare_dram_parameter("ident4x", [128, H], cdt, isOutput=False)
    xth_d = nc.declare_dram_parameter("xth", [B_LOC, 3, 128, N], cdt, isOutput=False)
    out_d = nc.declare_dram_parameter("out", [B_LOC, C], f32, isOutput=True)

    EXP = mybir.ActivationFunctionType.Exp

    with tile.TileContext(nc) as tc:
        with (
            tc.tile_pool(name="const", bufs=1) as const,
            tc.tile_pool(name="xp", bufs=3) as xp,
            tc.tile_pool(name="xtp", bufs=3) as xtp,
            tc.tile_pool(name="wp", bufs=4) as wpool,
            tc.tile_pool(name="small", bufs=2) as small,
            tc.tile_pool(name="ps_xt", bufs=2, space="PSUM") as ps_xt,
            tc.tile_pool(name="ps_dots", bufs=2, space="PSUM") as ps_dots,
            tc.tile_pool(name="ps_wt", bufs=2, space="PSUM") as ps_wt,
            tc.tile_pool(name="ps_acc", bufs=1, space="PSUM") as ps_acc,
        ):
            ident = const.tile([128, 128], cdt)
            nc.sync.dma_start(ident[:, :], id_d[:, :])
            ident4x = const.tile([128, H], cdt)
            nc.sync.dma_start(ident4x[:, :], id4_d[:, :])
            wkT = const.tile([128, CJ, 32], cdt)
            nc.sync.dma_start(
                wkT[:, :, :], wk_d.ap().rearrange("(j p) h -> p j h", p=128)
            )

            pooled_all = const.tile([H, B_LOC, C], cdt)
            zT = const.tile([128, CJ, B_LOC], cdt)
            pT = const.tile([128, CJ, B_LOC, H], cdt)
            wvT = const.tile([128, CJ, C], cdt)
            wpT = const.tile([128, CJ, C], cdt)
            bias = const.tile([B_LOC, C], f32)

            x_flat = x_d.ap()
            xth_ap = xth_d.ap()

            for b in range(B_LOC):
                acc_lo = ps_acc.tile([128, 512], f32, tag="acc_lo")
                acc_hi = ps_acc.tile([128, C2 - 512], f32, tag="acc_hi")

                for t2 in range(N // DTILE):
                    r0 = b * N + t2 * DTILE
                    # natural x: partition p holds rows 8p..8p+7 (12KB descs)
                    x_sb = xp.tile([128, DSUB, C2], cdt, tag="x")
                    xq = nc.scalar if (t2 % 2 == 0) else nc.sync
                    xq.dma_start(
                        out=x_sb[:, :, :],
                        in_=x_flat[r0 : r0 + DTILE, :].rearrange(
                            "(p s) c -> p s c", s=DSUB
                        ),
                    )
                    xTh2 = xtp.tile([128, 3, DTILE], cdt, tag="xth")
                    xtq = nc.sync if (t2 % 2 == 0) else nc.scalar
                    xtq.dma_start(
                        out=xTh2[:, :, :],
                        in_=xth_ap[b, :, :, t2 * DTILE : (t2 + 1) * DTILE]
                        .rearrange("j p n -> p j n"),
                    )
                    if b == 0 and t2 == 0:
                        # epilogue weights: after first x tile so startup
                        # stays lean
                        nc.sync.dma_start(
                            wvT[:, :, :],
                            wv_d.ap().rearrange("(j p) e -> p j e", p=128),
                        )
                        nc.sync.dma_start(
                            wpT[:, :, :],
                            wp_d.ap().rearrange("(j p) e -> p j e", p=128),
                        )
                        bp_ap = bp_d.ap()
                        nc.gpsimd.dma_start(
                            out=bias[:, :],
                            in_=bass.AP(
                                tensor=bp_ap.tensor,
                                offset=bp_ap.offset,
                                ap=[[0, B_LOC], [1, C]],
                            ),
                        )

                    for half in range(DTILE // PTILE):
                        pt = t2 * 2 + half
                        sb = half * GRP
                        first = pt == 0
                        last = pt == NPT - 1

                        # xT: chunks 0..2 transposed on-chip (PE),
                        # chunks 3..5 pre-transposed on host, DMA'd here
                        xT_sb = xtp.tile([128, 3, PTILE], cdt, tag="xt")
                        for j in range(GRP):
                            xt_ps = ps_xt.tile([128, 3, 128], cdt, tag="xt")
                            for cj in range(3):
                                nc.tensor.transpose(
                                    xt_ps[:, cj, :],
                                    x_sb[:, sb + j, cj * 128 : (cj + 1) * 128],
                                    ident[:, :],
                                )
                            nc.vector.tensor_copy(
                                xT_sb[:, 0:3, j * 128 : (j + 1) * 128],
                                xt_ps[:, :, :],
                            )

                        # dots: 4 concurrent column groups, one per subtile
                        dots_ps = ps_dots.tile([128, 128], f32, tag="dots")
                        for cj in range(CJ):
                            for j in range(GRP):
                                if cj < 3:
                                    rhs = xT_sb[:, cj, j * 128 : (j + 1) * 128]
                                else:
                                    n0 = half * PTILE + j * 128
                                    rhs = xTh2[:, cj - 3, n0 : n0 + 128]
                                nc.tensor.matmul(
                                    dots_ps[32 * j : 32 * j + 32, :],
                                    wkT[:, cj, :],
                                    rhs,
                                    start=(cj == 0),
                                    stop=(cj == CJ - 1),
                                    tile_position=(0, 32 * j),
                                    skip_group_check=True,
                                )

                        # w = exp(dots)   [4 groups x H, 128]
                        w_sb = wpool.tile([128, 128], cdt, tag="w")
                        nc.scalar.activation(w_sb[:, :], dots_ps[:, :], EXP)

                        # compact the 4 groups to partitions 0..H, then
                        # transpose each [H, 128] block -> [128, H]
                        w_sbc = wpool.tile([H, PTILE], cdt, tag="wc")
                        for j in range(GRP):
                            cfn = nc.vector.tensor_copy if j % 2 == 0 else nc.scalar.copy
                            cfn(
                                w_sbc[:, j * 128 : (j + 1) * 128],
                                w_sb[32 * j : 32 * j + H, :],
                            )
                        wt_ps = ps_wt.tile([128, CJ * H], cdt, tag="wt")
                        for j in range(GRP):
                            nc.tensor.transpose(
                                wt_ps[:, j * H : (j + 1) * H],
                                w_sbc[:, j * 128 : (j + 1) * 128],
                                ident[:H, :H],
                            )
                        wT_sb = wpool.tile([128, GRP * H], cdt, tag="wts")
                        nc.vector.tensor_copy(wT_sb[:, :], wt_ps[:, 0 : GRP * H])

                        # pooled accumulation: 4 concurrent column groups
                        # (+ ones columns -> sum of weights)
                        for j in range(GRP):
                            nc.tensor.matmul(
                                acc_lo[32 * j : 32 * j + H, :],
                                wT_sb[:, j * H : (j + 1) * H],
                                x_sb[:, sb + j, 0:512],
                                start=first,
                                stop=last,
                                tile_position=(0, 32 * j),
                                skip_group_check=True,
                            )
                            nc.tensor.matmul(
                                acc_hi[32 * j : 32 * j + H, :],
                                wT_sb[:, j * H : (j + 1) * H],
                                x_sb[:, sb + j, 512:C2],
                                start=first,
                                stop=last,
                                tile_position=(0, 32 * j),
                                skip_group_check=True,
                            )

                # merge the 4 column-group partials (DVE reads at most one
                # PSUM operand per instruction -> copy then chained adds)
                t0_lo = small.tile([H, 512], f32, tag="m0lo")
                t1_lo = small.tile([H, 512], f32, tag="m1lo")
                lo_m = small.tile([H, 512], f32, tag="mlo")
                nc.vector.tensor_copy(t0_lo[:, :], acc_lo[0:H, :])
                nc.vector.tensor_add(t1_lo[:, :], t0_lo[:, :], acc_lo[32 : 32 + H, :])
                nc.vector.tensor_add(t0_lo[:, :], t1_lo[:, :], acc_lo[64 : 64 + H, :])
                nc.vector.tensor_add(lo_m[:, :], t0_lo[:, :], acc_lo[96 : 96 + H, :])
                t0_hi = small.tile([H, C2 - 512], f32, tag="m0hi")
                t1_hi = small.tile([H, C2 - 512], f32, tag="m1hi")
                hi_m = small.tile([H, C2 - 512], f32, tag="mhi")
                nc.vector.tensor_copy(t0_hi[:, :], acc_hi[0:H, :])
                nc.vector.tensor_add(t1_hi[:, :], t0_hi[:, :], acc_hi[32 : 32 + H, :])
                nc.vector.tensor_add(t0_hi[:, :], t1_hi[:, :], acc_hi[64 : 64 + H, :])
                nc.vector.tensor_add(hi_m[:, :], t0_hi[:, :], acc_hi[96 : 96 + H, :])

                # normalize: pooled = acc / sumw  (sumw at ones col C-512)
                recip = small.tile([H, 1], f32, tag="recip")
                nc.vector.reciprocal(recip[:, :], hi_m[:, C - 512 : C - 511])
                nc.vector.tensor_scalar_mul(
                    pooled_all[:, b, 0:512], lo_m[:, :], recip[:, :]
                )
                nc.vector.tensor_scalar_mul(
                    pooled_all[:, b, 512:C], hi_m[:, 0 : C - 512], recip[:, :]
                )

                # per-batch epilogue: pooled -> pooledT -> pT[:, :, b]
                pT_ps = ps_wt.tile([128, CJ * H], cdt, tag="wt")
                for cj in range(CJ):
                    nc.tensor.transpose(
                        pT_ps[:, cj * H : (cj + 1) * H],
                        pooled_all[:, b, cj * 128 : (cj + 1) * 128],
                        ident[:H, :H],
                    )
                nc.vector.tensor_copy(
                    pT[:, :, b, :],
                    pT_ps[:, 0 : CJ * H].rearrange("p (j h) -> p j h", j=CJ),
                )
            # ---- end epilogue: z = per-head pooled @ Wv.T ----
            for h in range(H):
                zT_ps = ps_dots.tile([DH, B_LOC], f32, tag="dots")
                for cj in range(CJ):
                    nc.tensor.matmul(
                        zT_ps[:, :],
                        wvT[:, cj, h * DH : (h + 1) * DH],
                        pT[:, cj, :, h],
                        start=(cj == 0),
                        stop=(cj == CJ - 1),
                    )
                # scatter zT_ps rows (global hd = 96h+d) into zT chunks
                done = 0
                while done < DH:
                    g = h * DH + done
                    j, off = g // 128, g % 128
                    take = min(DH - done, 128 - off, 32)
                    nc.vector.tensor_copy(
                        zT[off : off + take, j, :],
                        zT_ps[done : done + take, :],
                    )
                    done += take

            # out = zT.T @ WprojT + bias
            o_lo = ps_acc.tile([B_LOC, 512], f32, tag="acc_lo")
            o_hi = ps_acc.tile([B_LOC, C - 512], f32, tag="acc_hi")
            for cj in range(CJ):
                nc.tensor.matmul(
                    o_lo[:, :],
                    zT[:, cj, :],
                    wpT[:, cj, 0:512],
                    start=(cj == 0),
                    stop=(cj == CJ - 1),
                )
                nc.tensor.matmul(
                    o_hi[:, :],
                    zT[:, cj, :],
                    wpT[:, cj, 512:C],
                    start=(cj == 0),
                    stop=(cj == CJ - 1),
                )
            out_sb = small.tile([B_LOC, C], f32, tag="osb")
            nc.vector.tensor_add(out_sb[:, 0:512], o_lo[:, :], bias[:, 0:512])
            nc.vector.tensor_add(out_sb[:, 512:C], o_hi[:, :], bias[:, 512:C])
            nc.sync.dma_start(out_d[:, :], out_sb[:, :])

    nc.compile()
    return nc


_NC_CACHE = None


def prepare_in_maps(x, queries, Wq, Wkv, Wproj, bproj):
    import ml_dtypes

    np_cdt = ml_dtypes.bfloat16

    x = np.asarray(x, dtype=np.float32)
    queries = np.asarray(queries, dtype=np.float32)
    Wq = np.asarray(Wq, dtype=np.float32)
    Wkv = np.asarray(Wkv, dtype=np.float32)
    Wproj = np.asarray(Wproj, dtype=np.float32)
    bproj = np.asarray(bproj, dtype=np.float32)

    # host-side weight folding (O(C^2), negligible vs O(B*N*C) device work)
    q = (queries @ Wq.T).reshape(H, DH)                     # [H, DH]
    Wk = Wkv[:C].reshape(H, DH, C)                          # [H, DH, C]
    wk_eff = np.einsum("hd,hdc->hc", q, Wk)                 # [H, C]
    wkT = np.zeros((C, 32), dtype=np.float32)               # [C, 32] padded
    wkT[:, :H] = wk_eff.T
    wkT = wkT.astype(np_cdt)
    wvT = np.ascontiguousarray(Wkv[C:].T).astype(np_cdt)    # [C, C] (c, hd)
    wpT = np.ascontiguousarray(Wproj.T).astype(np_cdt)      # [C, C] (hd, e)
    ident = np.eye(128, dtype=np.float32).astype(np_cdt)
    ident4x = np.zeros((128, H), dtype=np.float32)
    for j in range(GRP):
        ident4x[32 * j : 32 * j + H, :] = np.eye(H)
    ident4x = ident4x.astype(np_cdt)

    xb = x.astype(np_cdt)                                   # [B, N, C]
    in_maps = []
    for core in range(N_CORES):
        xc = xb[core * B_LOC : (core + 1) * B_LOC]          # [B_LOC, N, C]
        xs1 = np.empty((B_LOC * N, C2), dtype=np_cdt)
        xs1[:, :C] = xc.reshape(B_LOC * N, C)
        xs1[:, C:] = 1.0
        # xth[b, cj, c, pt*512 + j*128 + m] = x[b, row(pt, j, m), 384+cj*128+c]
        # with row = 1024*(pt//2) + 8m + 4*(pt%2) + j
        v = xc[:, :, 384:].reshape(B_LOC, 4, 128, 2, 4, 3, 128)
        xth = np.ascontiguousarray(v.transpose(0, 5, 6, 1, 3, 4, 2))
        in_maps.append(
            {
                "x": xs1,
                "xth": xth.reshape(B_LOC, 3, 128, N),
                "wkT": wkT,
                "wvT": wvT,
                "wpT": wpT,
                "bproj": bproj,
                "ident": ident,
                "ident4x": ident4x,
            }
        )
    return in_maps


def kernel(x, queries, Wq, Wkv, Wproj, bproj):
    global _NC_CACHE
    in_maps = prepare_in_maps(x, queries, Wq, Wkv, Wproj, bproj)
    if _NC_CACHE is None:
        _NC_CACHE = build_graph()
    nc = _NC_CACHE

    from concourse.bass_utils import run_bass_kernel_spmd

    res = run_bass_kernel_spmd(nc, in_maps, core_ids=list(range(N_CORES)))
    out = np.stack([res.results[i]["out"] for i in range(N_CORES)])  # [8,4,C]
    return out.reshape(B, 1, C).astype(np.float32)


# revision 21
# speedup vs baseline: 1.0066x; 1.0066x over previous
"""Trainium2 Bass kernel for single-query pooling attention.

Reference computation (B=32, N=4096, C=768, H=8, DH=96):
    q = (queries @ Wq.T).reshape(H, DH)
    k/v from x @ Wkv.T ; dots = q.k ; attn = softmax_n(dots)
    out = Wproj(attn-weighted sum of v) + bproj     -> [B, 1, C]

Algebraic reduction (never materializes k/v):
    wk_eff[h,c] = sum_d q[h,d] * Wkv[h*DH+d, c]         (host, tiny)
    dots[n,h]   = x[n,:] @ wk_eff[h,:]                  (PE, from xT)
    w = exp(dots)  (no max subtraction: |dots| <~ 20, safe in f32)
    pooled[h,c] = sum_n w[n,h] x[n,c] ; sumw[h] = sum_n w[n,h]
                  (one PE accumulation using ones-columns on x)
    z[hd]  = per-head pooled @ Wv.T ; out = z @ Wproj.T + bproj

The dots matmul needs x with channels on partitions (xT) while the
pooled matmul needs rows on partitions — the host supplies both
layouts in bf16 (same total bytes as one f32 copy), so the PE never
transposes x on-chip.

Sharding: pure data-parallel over batch, 4 batches per core, 8 cores.
No collectives needed.
"""

import sys

sys.path.insert(0, "/opt/trn_rl_repo")

import numpy as np

import concourse.bass as bass
import concourse.tile as tile
from concourse import bacc, mybir

B, N, C, H = 32, 4096, 768, 8
DH = C // H
N_CORES = 8
B_LOC = B // N_CORES          # 4 batches per core
TILE = 512                    # n rows per tile
SUB = TILE // 128             # 4 sub-tiles of 128 rows
NT = N // TILE                # 8 tiles per batch
CJ = C // 128                 # 6 c-chunks
C2 = C + 2                    # x padded with 4 ones columns (even psum mms)

COMPUTE = "bf16"              # "f32r" or "bf16"


def _cdt():
    return mybir.dt.float32r if COMPUTE == "f32r" else mybir.dt.bfloat16


def _np_cdt():
    if COMPUTE == "f32r":
        return np.float32
    import ml_dtypes

    return ml_dtypes.bfloat16


def build_graph():
    cdt = _cdt()
    f32 = mybir.dt.float32

    nc = bacc.Bacc("TRN2", target_bir_lowering=False, debug=False)

    x_d = nc.declare_dram_parameter("x", [B_LOC * N, C2], cdt, isOutput=False)
    xt_d = nc.declare_dram_parameter("xT", [B_LOC, C, N], cdt, isOutput=False)
    wk_d = nc.declare_dram_parameter("wkT", [C, H], cdt, isOutput=False)
    wv_d = nc.declare_dram_parameter("wvT", [C, C], cdt, isOutput=False)
    wp_d = nc.declare_dram_parameter("wpT", [C, C], cdt, isOutput=False)
    bp_d = nc.declare_dram_parameter("bproj", [C], f32, isOutput=False)
    id_d = nc.declare_dram_parameter("ident", [128, 128], cdt, isOutput=False)
    out_d = nc.declare_dram_parameter("out", [B_LOC, C], f32, isOutput=True)

    EXP = mybir.ActivationFunctionType.Exp

    with tile.TileContext(nc) as tc:
        with (
            tc.tile_pool(name="const", bufs=1) as const,
            tc.tile_pool(name="xp", bufs=3) as xp,
            tc.tile_pool(name="xtbig", bufs=16) as xtbig,
            tc.tile_pool(name="wp", bufs=3) as wpool,
            tc.tile_pool(name="small", bufs=4) as small,
            tc.tile_pool(name="ps_dots", bufs=3, space="PSUM") as ps_dots,
            tc.tile_pool(name="ps_wt", bufs=2, space="PSUM") as ps_wt,
            tc.tile_pool(name="ps_acc", bufs=1, space="PSUM") as ps_acc,
        ):
            ident = const.tile([128, 128], cdt)
            nc.sync.dma_start(ident[:, :], id_d[:, :])
            wkT = const.tile([128, CJ, H], cdt)
            nc.sync.dma_start(
                wkT[:, :, :], wk_d.ap().rearrange("(j p) h -> p j h", p=128)
            )

            pooled_all = const.tile([H, B_LOC, C], cdt)
            zT = const.tile([128, CJ, B_LOC], cdt)
            pT = const.tile([128, CJ, B_LOC, H], cdt)
            wvT = const.tile([128, CJ, C], cdt)
            wpT = const.tile([128, CJ, C], cdt)
            bias = const.tile([B_LOC, C], f32)

            x_flat = x_d.ap()
            xt_ap = xt_d.ap()

            NH = N // 2

            def load_xt_half(b, half, xts):
                for cj in range(CJ):
                    xt_sb = xtbig.tile([128, NH], cdt, tag="xtbig")
                    nc.sync.dma_start(
                        xt_sb[:, :],
                        xt_ap[
                            b,
                            cj * 128 : (cj + 1) * 128,
                            half * NH : (half + 1) * NH,
                        ],
                    )
                    xts.append(xt_sb)

            for b in range(B_LOC):
                # resident transposed x: 2 halves x 6 chunks x [128, N/2]
                xts = []
                load_xt_half(b, 0, xts)
                load_xt_half(b, 1, xts)
                if b == 0:
                    # epilogue weights: after batch-0 xT so startup stays lean
                    nc.sync.dma_start(
                        wvT[:, :, :],
                        wv_d.ap().rearrange("(j p) e -> p j e", p=128),
                    )
                    nc.sync.dma_start(
                        wpT[:, :, :],
                        wp_d.ap().rearrange("(j p) e -> p j e", p=128),
                    )
                    bp_ap = bp_d.ap()
                    nc.gpsimd.dma_start(
                        out=bias[:, :],
                        in_=bass.AP(
                            tensor=bp_ap.tensor,
                            offset=bp_ap.offset,
                            ap=[[0, B_LOC], [1, C]],
                        ),
                    )

                acc_lo = ps_acc.tile([H, 512], f32, tag="acc_lo")
                acc_hi = ps_acc.tile([H, C2 - 512], f32, tag="acc_hi")

                for t in range(NT):
                    r0 = b * N + t * TILE
                    # natural x: partition p holds rows 4p..4p+3 (6KB descs)
                    x_sb = xp.tile([128, SUB, C2], cdt, tag="x")
                    nc.scalar.dma_start(
                        out=x_sb[:, :, :],
                        in_=x_flat[r0 : r0 + TILE, :].rearrange(
                            "(p s) c -> p s c", s=SUB
                        ),
                    )

                    # dots[h, n-col] accumulated over c-chunks
                    dots = ps_dots.tile([H, TILE], f32, tag="dots")
                    for cj in range(CJ):
                        nc.tensor.matmul(
                            dots[:, :],
                            wkT[:, cj, :],
                            xts[(t // 4) * CJ + cj][
                                :, (t % 4) * TILE : (t % 4 + 1) * TILE
                            ],
                            start=(cj == 0),
                            stop=(cj == CJ - 1),
                        )

                    # w = exp(dots)  [H, TILE]
                    w_sb = wpool.tile([H, TILE], cdt, tag="w")
                    nc.scalar.activation(w_sb[:, :], dots[:, :], EXP)

                    # transpose w -> [n, h] blocks
                    wT_ps = ps_wt.tile([128, CJ * H], cdt, tag="wt")
                    for s in range(SUB):
                        nc.tensor.transpose(
                            wT_ps[:, s * H : (s + 1) * H],
                            w_sb[:, s * 128 : (s + 1) * 128],
                            ident[:H, :H],
                        )
                    wT_sb = wpool.tile([128, SUB * H], cdt, tag="wts")
                    nc.vector.tensor_copy(wT_sb[:, :], wT_ps[:, 0 : SUB * H])

                    # pooled accumulation (+ ones columns -> sum of weights)
                    for s in range(SUB):
                        first = t == 0 and s == 0
                        last = t == NT - 1 and s == SUB - 1
                        nc.tensor.matmul(
                            acc_lo[:, :],
                            wT_sb[:, s * H : (s + 1) * H],
                            x_sb[:, s, 0:512],
                            start=first,
                            stop=last,
                        )
                        nc.tensor.matmul(
                            acc_hi[:, :],
                            wT_sb[:, s * H : (s + 1) * H],
                            x_sb[:, s, 512:C2],
                            start=first,
                            stop=last,
                        )

                # normalize: pooled = acc / sumw   (sumw at ones col C-512)
                recip = small.tile([H, 1], f32, tag="recip")
                nc.vector.reciprocal(recip[:, :], acc_hi[:, C - 512 : C - 511])
                nc.vector.tensor_scalar_mul(
                    pooled_all[:, b, 0:512], acc_lo[:, :], recip[:, :]
                )
                nc.vector.tensor_scalar_mul(
                    pooled_all[:, b, 512:C], acc_hi[:, 0 : C - 512], recip[:, :]
                )

                # per-batch epilogue: pooled -> pooledT -> zT[:, :, b]
                pT_ps = ps_wt.tile([128, CJ * H], cdt, tag="wt")
                for cj in range(CJ):
                    nc.tensor.transpose(
                        pT_ps[:, cj * H : (cj + 1) * H],
                        pooled_all[:, b, cj * 128 : (cj + 1) * 128],
                        ident[:H, :H],
                    )
                nc.vector.tensor_copy(
                    pT[:, :, b, :],
                    pT_ps[:, 0 : CJ * H].rearrange("p (j h) -> p j h", j=CJ),
                )
            # ---- end epilogue: z = per-head pooled @ Wv.T ----
            for h in range(H):
                zT_ps = ps_dots.tile([DH, B_LOC], f32, tag="dots")
                for cj in range(CJ):
                    nc.tensor.matmul(
                        zT_ps[:, :],
                        wvT[:, cj, h * DH : (h + 1) * DH],
                        pT[:, cj, :, h],
                        start=(cj == 0),
                        stop=(cj == CJ - 1),
                    )
                # scatter zT_ps rows (global hd = 96h+d) into zT chunks
                done = 0
                while done < DH:
                    g = h * DH + done
                    j, off = g // 128, g % 128
                    take = min(DH - done, 128 - off, 32)
                    nc.vector.tensor_copy(
                        zT[off : off + take, j, :],
                        zT_ps[done : done + take, :],
                    )
                    done += take

            # out = zT.T @ WprojT + bias
            o_lo = ps_acc.tile([B_LOC, 512], f32, tag="acc_lo")
            o_hi = ps_acc.tile([B_LOC, C - 512], f32, tag="acc_hi")
            for cj in range(CJ):
                nc.tensor.matmul(
                    o_lo[:, :],
                    zT[:, cj, :],
                    wpT[:, cj, 0:512],
                    start=(cj == 0),
                    stop=(cj == CJ - 1),
                )
                nc.tensor.matmul(
                    o_hi[:, :],
                    zT[:, cj, :],
                    wpT[:, cj, 512:C],
                    start=(cj == 0),
                    stop=(cj == CJ - 1),
                )
            out_sb = small.tile([B_LOC, C], f32, tag="osb")
            nc.vector.tensor_add(out_sb[:, 0:512], o_lo[:, :], bias[:, 0:512])
            nc.vector.tensor_add(out_sb[:, 512:C], o_hi[:, :], bias[:, 512:C])
            nc.sync.dma_start(out_d[:, :], out_sb[:, :])

    nc.compile()
    return nc


_NC_CACHE = None


def prepare_in_maps(x, queries, Wq, Wkv, Wproj, bproj):
    x = np.asarray(x, dtype=np.float32)
    queries = np.asarray(queries, dtype=np.float32)
    Wq = np.asarray(Wq, dtype=np.float32)
    Wkv = np.asarray(Wkv, dtype=np.float32)
    Wproj = np.asarray(Wproj, dtype=np.float32)
    bproj = np.asarray(bproj, dtype=np.float32)

    # host-side weight folding (O(C^2), negligible vs O(B*N*C) device work)
    q = (queries @ Wq.T).reshape(H, DH)                     # [H, DH]
    Wk = Wkv[:C].reshape(H, DH, C)                          # [H, DH, C]
    wk_eff = np.einsum("hd,hdc->hc", q, Wk)                 # [H, C]
    np_cdt = _np_cdt()
    wkT = np.ascontiguousarray(wk_eff.T).astype(np_cdt)     # [C, H]
    wvT = np.ascontiguousarray(Wkv[C:].T).astype(np_cdt)    # [C, C] (c, hd)
    wpT = np.ascontiguousarray(Wproj.T).astype(np_cdt)      # [C, C] (hd, e)
    ident = np.eye(128, dtype=np.float32).astype(np_cdt)

    xb = x.astype(np_cdt)                                   # [B, N, C]
    in_maps = []
    for core in range(N_CORES):
        xc = xb[core * B_LOC : (core + 1) * B_LOC]          # [B_LOC, N, C]
        xs1 = np.empty((B_LOC * N, C2), dtype=np_cdt)
        xs1[:, :C] = xc.reshape(B_LOC * N, C)
        xs1[:, C:] = 1.0
        # xT[b, c, t*512 + s*128 + q] = x[b, 512t + 4q + s, c]
        v = xc.reshape(B_LOC, NT, 128, SUB, C)              # [b, t, q, s, c]
        xT = np.ascontiguousarray(
            v.transpose(0, 4, 1, 3, 2)                      # [b, c, t, s, q]
        ).reshape(B_LOC, C, N)
        in_maps.append(
            {
                "x": xs1,
                "xT": xT,
                "wkT": wkT,
                "wvT": wvT,
                "wpT": wpT,
                "bproj": bproj,
                "ident": ident,
            }
        )
    return in_maps


def kernel(x, queries, Wq, Wkv, Wproj, bproj):
    global _NC_CACHE
    in_maps = prepare_in_maps(x, queries, Wq, Wkv, Wproj, bproj)
    if _NC_CACHE is None:
        _NC_CACHE = build_graph()
    nc = _NC_CACHE

    from concourse.bass_utils import run_bass_kernel_spmd

    res = run_bass_kernel_spmd(nc, in_maps, core_ids=list(range(N_CORES)))
    out = np.stack([res.results[i]["out"] for i in range(N_CORES)])  # [8,4,C]
    return out.reshape(B, 1, C).astype(np.float32)

